# revision 25
# baseline (speedup 1.0000x reference)
"""Causal self-attention (B=4, S=2048, D=1024, H=16) on 8 Trainium2 NeuronCores.

Sharding: core c handles batch b = c // 2 and head-group g = c % 2
(8 heads, 512 of the 1024 output dims).  Data parallel over B, tensor
parallel over heads — attention is embarrassingly parallel over (b, h).

Per-core device program (identical on all cores, SPMD with different data):
  1. Projections: QT/KT in [d, q] layout (d on partitions), V in natural
     [k, d] layout with a ones-column appended (so the P@V matmul also
     produces the softmax denominator as an extra output row).
     All matmul operands fp16 (host-cast), accumulation in fp32 PSUM.
  2. Attention per head-pair: scoresT[k, q] tiles via row-packed (d=64)
     matmuls for two heads concurrently; exp on ScalarE with per-partition
     bias = -SHIFT + attention-mask bias (scale 1/sqrt(64) folded into Wq
     host-side); causal mask via tile skipping + one triangular 128x128
     multiply on diagonal tiles; PV accumulates ctxT[d(+1), q] over k-tiles.
  3. Unnormalized ctxT and the denominator row are DMA'd out; the host
     divides and re-assembles the [B, S, D] output.
"""

import numpy as np

B, S, D, H, HD = 4, 2048, 1024, 16, 64
DC = 512          # output dims per core (8 heads)
P = 128
NQC = S // 512    # q-chunks of 512
NKT = S // P      # k-tiles of 128
SHIFT = 8.0       # exp(score - SHIFT); cancels in the normalization
NEG = -30000.0    # attention-mask "minus infinity"
# fp8 hi/lo projection scales (powers of two; fold out exactly).  X is
# scaled by 2^SXH before the hi/lo split, weights by 2^SW*; the product
# scale is divided out in the PSUM->SBUF stage (Q/K) or carried through
# v65 AND the denominator ones-row so it cancels in the host division (V).
SXH, SWQ, SWK, SWV = 5, 10, 6, 7
VSC = float(2 ** (SXH + SWV))
F8MAX = 240.0

_PROG = None


def _emit_body(nc, t, pools):
    """One full compute pass: projections + attention + output DMA."""
    from concourse import mybir
    from concourse.bass import ds, ts

    f32 = mybir.dt.float32
    f16 = mybir.dt.float16
    EXP = mybir.ActivationFunctionType.Exp
    MULT = mybir.AluOpType.mult
    ADD = mybir.AluOpType.add
    epool, opool, psp, pss, psc = pools
    import os as _osm
    fp8 = _osm.environ.get("K_FP8PROJ", "0") == "1"
    DRM = mybir.MatmulPerfMode.DoubleRow

    def qk_mms(wkey, dt, qsl):
        """Matmul operand sequence for a Q/K projection unit (output
        [128 dims, 512 q] accumulated in one PSUM group)."""
        if not fp8:
            w = t[wkey]
            return [(w[:, s, ts(dt, P)], t["ht"][:, s, qsl], None)
                    for s in range(8)]
        # fp8 hi/lo: x@W ~= xh@Wh + xl@Wh + xh@Wl, DoubleRow over s-pairs
        w = t[wkey]
        out = []
        for tt in range(4):
            sl = slice(2 * tt, 2 * tt + 2)
            for hw_, hx in ((0, 0), (1, 0), (0, 1)):
                out.append((w[:, hw_, sl, ts(dt, P)],
                            t["htx"][:, hx, sl, qsl], DRM))
        return out

    def v_mms(kt_i):
        ksl = ds(kt_i * P, P)
        if not fp8:
            return [(t["ht"][:, s, ksl], t["wvt"][:, s, :], None)
                    for s in range(8)]
        out = []
        for tt in range(4):
            sl = slice(2 * tt, 2 * tt + 2)
            for hx, hw_ in ((0, 0), (1, 0), (0, 1)):
                out.append((t["htx"][:, hx, sl, ksl],
                            t["wvx"][:, hw_, sl, :], DRM))
        return out

    def qk_fin(dst, bt, dt, qsl, scale):
        def fin(pp):
            if fp8:
                nc.vector.tensor_scalar(
                    dst[:, dt, qsl], pp[:], scale, bt[:, dt : dt + 1],
                    MULT, ADD,
                )
            else:
                nc.vector.tensor_scalar_add(
                    dst[:, dt, qsl], pp[:], bt[:, dt : dt + 1]
                )
        return fin

    def v_fin(kt_i):
        def fin(pp):
            nc.vector.tensor_tensor(
                t["v65"][:, kt_i, :, 0:64],
                pp[:].rearrange("p (h d) -> p h d", h=8),
                t["bvr_t"][:].rearrange("p (h d) -> p h d", h=8),
                ADD,
            )
        return fin

    QSC, KSC = 2.0 ** (-SXH - SWQ), 2.0 ** (-SXH - SWK)

    def unit_defs(c):
        """(key, mm-list, fin) for the 12 projection units of chunk c."""
        qsl = ds(c * 512, 512)
        wq = "wqx" if fp8 else "wqt"
        wk = "wkx" if fp8 else "wkt"
        defs = []
        for dt in range(4):
            defs.append((("K", dt), qk_mms(wk, dt, qsl),
                         qk_fin(t["ktt"], t["bk_t"], dt, qsl, KSC)))
            defs.append((("V", dt), v_mms(4 * c + dt), v_fin(4 * c + dt)))
            defs.append((("Q", dt), qk_mms(wq, dt, qsl),
                         qk_fin(t["qt"], t["bq_t"], dt, qsl, QSC)))
        return defs

    def run_unit(mms, fin):
        pp = psp.tile([P, 2, 512], f32, tag="sc", name="pp")[:, 0, :] \
            if psp is pss else psp.tile([P, 512], f32, tag="proj", name="pp")
        n = len(mms)
        for i, (lhsT, rhs, pm) in enumerate(mms):
            nc.tensor.matmul(pp[:], lhsT, rhs, start=(i == 0),
                             stop=(i == n - 1), perf_mode=pm)
        fin(pp)

    def proj_units(c):
        """Projection work for q-chunk c as a list of callables (one PSUM
        group each) so they can be interleaved into the attention stream."""
        return [
            (lambda mms=mms, fin=fin: run_unit(mms, fin))
            for _key, mms, fin in unit_defs(c)
        ]

    def proj_steps_by_unit(c):
        """Projection work for q-chunk c as fine-grained steps (~2-3 matmuls
        each, ~320-430ns of PE), keyed by unit ('K'|'Q'|'V', dt) so the
        global scheduler can place each unit's 4 steps before its deadline."""
        units = {}
        for key, mms, fin in unit_defs(c):
            cell = {}
            per = len(mms) // 4

            def step(i, mms=mms, fin=fin, cell=cell, per=per):
                def run():
                    if i == 0:
                        cell["pp"] = psp.tile(
                            [P, 512], f32, tag="proj", name="pp"
                        )
                    pp = cell["pp"]
                    n = len(mms)
                    for j in range(per * i, per * (i + 1)):
                        lhsT, rhs, pm = mms[j]
                        nc.tensor.matmul(
                            pp[:], lhsT, rhs, start=(j == 0),
                            stop=(j == n - 1), perf_mode=pm,
                        )
                    if i == 3:
                        fin(pp)
                return run

            units[key] = [step(i) for i in range(4)]
        return units

    import os as _os
    ndr = int(_os.environ.get("K_DR_PROBE", "0"))
    if ndr:
        # TIMING PROBE: ndr dummy fp8 DoubleRow matmuls (2x128 contraction,
        # 512 cols each) inside the repeated body, so the slope measures them
        wdr = psp.tile([P, 512], f32, tag="proj", name="wdr") \
            if psp is not pss else pss.tile([P, 2, 512], f32, tag="sc")[:, 0, :]
        for i in range(ndr):
            nc.tensor.matmul(
                wdr[:], t["scr8"][:, :, 0:P], t["scr8"][:, :, :],
                start=(i == 0), stop=(i == ndr - 1), perf_mode=DRM,
            )
    order = _os.environ.get("K_ORDER", "global")

    # prologue: emit chunk-0 projections so early units match DMA arrival
    # order (wkt lands first, then wqt, then wvt) — K1/Q1 fill the PE while
    # wvt is still in flight; attention needs K0,Q0,V0..V3 before pair 0.
    u0 = proj_units(0)  # [K0,V0,Q0, K1,V1,Q1, K2,V2,Q2, K3,V3,Q3]
    for i in (0, 2, 3, 5, 1, 4, 7, 10, 6, 8, 9, 11):
        u0[i]()

    if order == "global":
        # Deadline-aware placement of the remaining 144 projection steps
        # over the 160 attention kt-slots.  Late-needed chunk-3 units are
        # reserved for chunk-3's attention (which otherwise has no PE
        # filler and runs ACT-limited); early chunks pop 2 steps/slot to
        # absorb the surplus without boundary drains.  Unit (c,'K'/'Q',dt)
        # must finish before chunk-c pair-dt's first kt (K: before kt 4c);
        # unit (c,'V',dt) before kt 4c+dt of chunk c's pair 0.
        SCHED = {
            0: [(1, "K", 0), (1, "Q", 0), (1, "V", 0), (1, "V", 1),
                (1, "V", 2), (1, "V", 3)],
            1: [(1, "K", 1), (1, "Q", 1), (1, "K", 2), (1, "Q", 2),
                (1, "K", 3), (1, "Q", 3), (2, "K", 0), (2, "Q", 0),
                (2, "V", 0), (2, "V", 1), (2, "V", 2), (2, "V", 3)],
            2: [(2, "K", 1), (2, "Q", 1), (2, "K", 2), (2, "Q", 2),
                (2, "K", 3), (2, "Q", 3), (3, "Q", 0), (3, "V", 0),
                (3, "V", 1), (3, "K", 0), (3, "V", 2), (3, "V", 3)],
            3: [(3, "Q", 1), (3, "K", 1), (3, "Q", 2), (3, "K", 2),
                (3, "Q", 3), (3, "K", 3)],
        }
        all_steps = {c: proj_steps_by_unit(c) for c in range(1, NQC)}

        def chunk_queue(c):
            return [s for key in SCHED[c] for s in all_steps[key[0]][key[1:]]]
    else:  # "fine": original schedule — next chunk's steps, 1 per kt
        def chunk_queue(c):
            return (
                [s for u in proj_steps_by_unit(c + 1).values() for s in u]
                if c + 1 < NQC else []
            )

    for c in range(NQC):
        qsl = ds(c * 512, 512)
        queue = chunk_queue(c)
        # attention for q-chunk c, all 4 head-pairs
        nkt = 4 * c + 4
        n_slots = 4 * nkt
        slot = 0
        pvpack = _os.environ.get("K_PVPACK", "0") == "1"
        for pr in range(4):
            cA = psc.tile([P, 512], f32, tag="ctx", name="cA")
            cB = cA if pvpack else psc.tile([P, 512], f32, tag="ctx", name="cB")
            for kt_i in range(nkt):
                # causal: q columns [0, off) of this (k-tile, q-chunk) pair
                # are fully masked — skip them in scores, exp and PV.
                j = kt_i - 4 * c
                off = 128 * j if j > 0 else 0
                qso = ds(c * 512 + off, 512 - off)
                pt = pss.tile([P, 2, 512], f32, tag="sc")
                nc.tensor.matmul(
                    pt[:, 0, off:512],
                    t["ktt"][0:64, pr, ds(kt_i * P, P)],
                    t["qt"][0:64, pr, qso],
                    start=True, stop=True, tile_position=(0, 0),
                )
                nc.tensor.matmul(
                    pt[:, 1, off:512],
                    t["ktt"][64:128, pr, ds(kt_i * P, P)],
                    t["qt"][64:128, pr, qso],
                    start=True, stop=True, tile_position=(64, 0),
                )
                e = epool.tile([P, 2, 512], f16, tag="e")
                kbias = t["kb_t"][:, kt_i : kt_i + 1]
                nc.scalar.activation(
                    e[:, :, off:512], pt[:, :, off:512], EXP, bias=kbias
                )
                if j >= 0:
                    nc.vector.tensor_tensor(
                        e[:, :, off : off + P],
                        e[:, :, off : off + P],
                        t["tri_t"][:][:, None, :].to_broadcast((P, 2, P)),
                        MULT,
                    )
                if pvpack:
                    # TIMING PROBE ONLY (wrong numerics: no denominator row):
                    # both heads' PV col-packed into disjoint PE quadrants
                    nc.tensor.matmul(
                        cA[0:64, off:512],
                        t["v65"][:, kt_i, 2 * pr, 0:64], e[:, 0, off:512],
                        start=(kt_i == 0), stop=(kt_i == nkt - 1),
                        tile_position=(0, 0),
                    )
                    nc.tensor.matmul(
                        cA[64:128, off:512],
                        t["v65"][:, kt_i, 2 * pr + 1, 0:64], e[:, 1, off:512],
                        start=(kt_i == 0), stop=(kt_i == nkt - 1),
                        tile_position=(0, 64),
                    )
                else:
                    nc.tensor.matmul(
                        cA[0:65, off:512],
                        t["v65"][:, kt_i, 2 * pr, :], e[:, 0, off:512],
                        start=(kt_i == 0), stop=(kt_i == nkt - 1),
                    )
                    nc.tensor.matmul(
                        cB[0:65, off:512],
                        t["v65"][:, kt_i, 2 * pr + 1, :], e[:, 1, off:512],
                        start=(kt_i == 0), stop=(kt_i == nkt - 1),
                    )
                slot += 1
                if queue:
                    # front-loaded pacing: enough steps per slot that the
                    # queue drains by the chunk's end (2/slot when over-full)
                    n_pop = min(len(queue),
                                max(1, -(-len(queue) // (n_slots - slot + 1))))
                    for _ in range(n_pop):
                        queue.pop(0)()
            oA = opool.tile([P, 512], f32, tag="o", name="oA")
            oB = opool.tile([P, 512], f32, tag="o", name="oB")
            if pvpack:
                nc.vector.tensor_copy(oA[0:65, :], cA[0:65, :])
                nc.vector.tensor_copy(oB[0:64, :], cA[64:128, :])
            else:
                nc.vector.tensor_copy(oA[0:65, :], cA[0:65, :])
                nc.vector.tensor_copy(oB[0:65, :], cB[0:65, :])
            nc.sync.dma_start(t["out_d"][2 * pr, :, qsl], oA[0:65, :])
            nc.sync.dma_start(t["out_d"][2 * pr + 1, :, qsl], oB[0:65, :])
        while queue:  # safety: finish any leftovers at the chunk boundary
            queue.pop(0)()


def _build(repeat=1):
    from contextlib import ExitStack

    import concourse.tile as tile
    from concourse import bacc, mybir

    f16, f32 = mybir.dt.float16, mybir.dt.float32

    nc = bacc.Bacc(
        "TRN2",
        target_bir_lowering=False,
        debug=False,
        enable_asserts=False,
        num_devices=8,
    )
    import os as _os0
    fp8 = _os0.environ.get("K_FP8PROJ", "0") == "1"
    f8 = mybir.dt.float8e4
    if fp8:
        htx_d = nc.dram_tensor("htx", [P, 2, 8, S], f8,
                               kind="ExternalInput").ap()
        wqx_d = nc.dram_tensor("wqx", [P, 2, 8, DC], f8,
                               kind="ExternalInput").ap()
        wkx_d = nc.dram_tensor("wkx", [P, 2, 8, DC], f8,
                               kind="ExternalInput").ap()
        wvx_d = nc.dram_tensor("wvx", [P, 2, 8, DC], f8,
                               kind="ExternalInput").ap()
    else:
        ht_d = nc.dram_tensor("ht", [D, S], f16, kind="ExternalInput").ap()
        wqt_d = nc.dram_tensor("wqt", [D, DC], f16, kind="ExternalInput").ap()
        wkt_d = nc.dram_tensor("wkt", [D, DC], f16, kind="ExternalInput").ap()
        wvt_d = nc.dram_tensor("wvt", [D, DC], f16, kind="ExternalInput").ap()
    bq_d = nc.dram_tensor("bq", [P, 4], f32, kind="ExternalInput").ap()
    bk_d = nc.dram_tensor("bk", [P, 4], f32, kind="ExternalInput").ap()
    bvr_d = nc.dram_tensor("bvr", [P, DC], f16, kind="ExternalInput").ap()
    kb_d = nc.dram_tensor("kbias", [P, NKT], f32, kind="ExternalInput").ap()
    tri_d = nc.dram_tensor("tri", [P, P], f16, kind="ExternalInput").ap()
    out_d = nc.dram_tensor("out", [8, 65, S], f32, kind="ExternalOutput").ap()

    import os as _os2

    with ExitStack() as ctx:
        tc = ctx.enter_context(tile.TileContext(nc))
        const = ctx.enter_context(tc.tile_pool(name="const", bufs=1))
        epool = ctx.enter_context(tc.tile_pool(name="epool", bufs=int(_os2.environ.get("K_EB", "8"))))
        opool = ctx.enter_context(tc.tile_pool(name="opool", bufs=4))
        import os as _os
        _sb = int(_os.environ.get("K_PSS_BUFS", "2"))
        _cb = int(_os.environ.get("K_PSC_BUFS", "2"))
        _pb = int(_os.environ.get("K_PSP_BUFS", "2"))
        pss = ctx.enter_context(tc.tile_pool(name="pss", bufs=_sb, space="PSUM"))
        if _pb:
            psp = ctx.enter_context(tc.tile_pool(name="psp", bufs=_pb, space="PSUM"))
        else:
            psp = pss  # projections share the scores pool slots
        psc = ctx.enter_context(tc.tile_pool(name="psc", bufs=_cb, space="PSUM"))

        t = dict(
            qt=const.tile([P, 4, S], f16, name="qt"),
            ktt=const.tile([P, 4, S], f16, name="ktt"),
            v65=const.tile([P, NKT, 8, 65], f16, name="v65"),
            bq_t=const.tile([P, 4], f32, name="bq_t"),
            bk_t=const.tile([P, 4], f32, name="bk_t"),
            bvr_t=const.tile([P, DC], f16, name="bvr_t"),
            kb_t=const.tile([P, NKT], f32, name="kb_t"),
            tri_t=const.tile([P, P], f16, name="tri_t"),
            out_d=out_d,
        )
        if fp8:
            t["htx"] = const.tile([P, 2, 8, S], f8, name="htx")
            t["wqx"] = const.tile([P, 2, 8, DC], f8, name="wqx")
            t["wkx"] = const.tile([P, 2, 8, DC], f8, name="wkx")
            t["wvx"] = const.tile([P, 2, 8, DC], f8, name="wvx")
        else:
            t["ht"] = const.tile([P, 8, S], f16, name="ht")
            t["wqt"] = const.tile([P, 8, DC], f16, name="wqt")
            t["wkt"] = const.tile([P, 8, DC], f16, name="wkt")
            t["wvt"] = const.tile([P, 8, DC], f16, name="wvt")
            ht_r = ht_d.rearrange("(o p) m -> p o m", p=P)
            wq_r = wqt_d.rearrange("(o p) m -> p o m", p=P)
            wk_r = wkt_d.rearrange("(o p) m -> p o m", p=P)
            wv_r = wvt_d.rearrange("(o p) m -> p o m", p=P)
        nc.sync.dma_start(t["bq_t"][:], bq_d)
        # warmup prep first: scr memset leads the DVE queue so the PE warmup
        # below isn't stuck behind DVE-issued weight DMAs
        warmpe = _os.environ.get("K_WARMPE", "1") == "1"
        if warmpe:
            scr = const.tile([P, 512], f16, name="scr")
            nc.vector.memset(scr[:], 0.0)
        # warmup exp so the ACT table load (~2.7us) hides behind startup DMAs
        warm = const.tile([P, 1], mybir.dt.float16, name="warm")
        nc.scalar.activation(
            warm[:], t["bq_t"][:, 0:1], mybir.ActivationFunctionType.Exp
        )
        if warmpe:
            # warm the PE (HAM clock gate) with dummy matmuls while the first
            # input DMAs are still in flight, so real work starts at 2.4 GHz
            wpt = (psp if psp is not pss else pss).tile(
                [P, 512], f32, tag="proj" if psp is not pss else "sc",
                name="wpt",
            )
            nwarm = int(_os.environ.get("K_WARM_N", "14"))
            for i in range(nwarm):
                nc.tensor.matmul(
                    wpt[:, 0:512], scr[:, 0:P], scr[:],
                    start=(i == 0), stop=(i == nwarm - 1),
                )
        if int(_os.environ.get("K_DR_PROBE", "0")):
            t["scr8"] = const.tile([P, 2, 512], mybir.dt.float8e4,
                                   name="scr8")
            nc.vector.memset(t["scr8"][:], 0.125)
        if _os.environ.get("K_DMA", "multi") == "multi":
            # Consumption-ordered loads spread over three engine queues
            # (issue cost is ~0.6-1us per DMA and serializes per engine;
            # the prologue consumes K0 -> Q0 -> V0..V3 first).
            if fp8:
                for s in range(8):
                    nc.sync.dma_start(t["wkx"][:, :, s, :], wkx_d[:, :, s, :])
                for s in range(8):
                    nc.sync.dma_start(t["wqx"][:, :, s, :], wqx_d[:, :, s, :])
            else:
                for s in range(8):
                    nc.sync.dma_start(t["wkt"][:, s, :], wk_r[:, s, :])
                for s in range(8):
                    nc.sync.dma_start(t["wqt"][:, s, :], wq_r[:, s, :])
            nc.sync.dma_start(t["bk_t"][:], bk_d)
            nc.sync.dma_start(t["bvr_t"][:], bvr_d)
            nc.sync.dma_start(t["kb_t"][:], kb_d)
            nc.sync.dma_start(t["tri_t"][:], tri_d)
            if fp8:
                for s in range(8):
                    nc.scalar.dma_start(t["htx"][:, :, s, 0:512],
                                        htx_d[:, :, s, 0:512])
                for s in range(8):
                    nc.scalar.dma_start(t["htx"][:, :, s, 512:2048],
                                        htx_d[:, :, s, 512:2048])
                for s in range(8):
                    nc.gpsimd.dma_start(t["wvx"][:, :, s, :],
                                        wvx_d[:, :, s, :])
            else:
                for s in range(8):
                    nc.scalar.dma_start(t["ht"][:, s, 0:512],
                                        ht_r[:, s, 0:512])
                for s in range(8):
                    nc.scalar.dma_start(t["ht"][:, s, 512:2048],
                                        ht_r[:, s, 512:2048])
                for s in range(8):
                    nc.gpsimd.dma_start(t["wvt"][:, s, :], wv_r[:, s, :])
        else:
            nc.sync.dma_start(t["bk_t"][:], bk_d)
            nc.sync.dma_start(t["bvr_t"][:], bvr_d)
            nc.sync.dma_start(t["kb_t"][:], kb_d)
            nc.sync.dma_start(t["tri_t"][:], tri_d)
            for s in range(8):
                nc.sync.dma_start(t["wqt"][:, s, :], wq_r[:, s, :])
                nc.sync.dma_start(t["ht"][:, s, 0:512], ht_r[:, s, 0:512])
            for s in range(8):
                nc.sync.dma_start(t["wkt"][:, s, :], wk_r[:, s, :])
                nc.sync.dma_start(t["wvt"][:, s, :], wv_r[:, s, :])
            for s in range(8):
                nc.sync.dma_start(t["ht"][:, s, 512:2048], ht_r[:, s, 512:2048])
        # fp8 path: v65 carries the 2^(SXH+SWV) product scale, so the
        # denominator ones-row must carry it too (cancels in host division)
        nc.vector.memset(t["v65"][:, :, :, 64:65], VSC if fp8 else 1.0)

        for _rep in range(repeat):
            _emit_body(nc, t, (epool, opool, psp, pss, psc))

    nc.compile()
    return nc


def _get_program():
    global _PROG
    if _PROG is None:
        _PROG = _build()
    return _PROG


def _split8(a, scale):
    """Scale, then split into fp8e4 hi + lo parts: a*scale ~= hi + lo."""
    import ml_dtypes

    f8 = ml_dtypes.float8_e4m3
    s = np.clip(a * scale, -F8MAX, F8MAX)
    hi = s.astype(f8)
    lo = np.clip(s - hi.astype(np.float32), -F8MAX, F8MAX).astype(f8)
    return hi, lo


def _pack8(hi, lo):
    """[D, M] hi/lo -> [P, 2, 8, M] (partition p holds rows s*128+p)."""
    m = hi.shape[1]
    return np.ascontiguousarray(
        np.stack([hi.reshape(8, P, m), lo.reshape(8, P, m)], axis=0)
        .transpose(2, 0, 1, 3)
    )


def prepare_in_maps(hidden_states, attention_mask, Wq, bq, Wk, bk, Wv, bv):
    import os as _os

    fp8 = _os.environ.get("K_FP8PROJ", "0") == "1"
    hidden_states = np.asarray(hidden_states, dtype=np.float32)
    attention_mask = np.asarray(attention_mask)
    Wq, bq = np.asarray(Wq, np.float32), np.asarray(bq, np.float32)
    Wk, bk = np.asarray(Wk, np.float32), np.asarray(bk, np.float32)
    Wv, bv = np.asarray(Wv, np.float32), np.asarray(bv, np.float32)
    tri = np.triu(np.ones((P, P), np.float16))  # tri[k, q] = 1 iff q >= k
    in_maps = []
    if fp8:
        hts = [_pack8(*_split8(hidden_states[b].T, 2.0 ** SXH))
               for b in range(B)]
    else:
        hts = [np.ascontiguousarray(hidden_states[b].T, dtype=np.float16)
               for b in range(B)]
    for c in range(8):
        b, g = divmod(c, 2)
        rows = slice(g * DC, (g + 1) * DC)
        am = np.asarray(attention_mask[b, 0, 0], np.float32)
        kbias = (np.where(am > 0, 0.0, NEG) - SHIFT).astype(np.float32)
        im = dict(
            bq=np.ascontiguousarray((bq[rows] * 0.125).reshape(4, P).T),
            bk=np.ascontiguousarray(bk[rows].reshape(4, P).T),
            kbias=np.ascontiguousarray(kbias.reshape(NKT, P).T),
            tri=tri,
        )
        if fp8:
            im["htx"] = hts[b]
            im["wqx"] = _pack8(*_split8((Wq[rows] * 0.125).T, 2.0 ** SWQ))
            im["wkx"] = _pack8(*_split8(Wk[rows].T, 2.0 ** SWK))
            im["wvx"] = _pack8(*_split8(Wv[rows].T, 2.0 ** SWV))
            im["bvr"] = np.broadcast_to(
                (bv[rows] * VSC).astype(np.float16), (P, DC)
            ).copy()
        else:
            im["ht"] = hts[b]
            im["wqt"] = np.ascontiguousarray((Wq[rows] * 0.125).T, np.float16)
            im["wkt"] = np.ascontiguousarray(Wk[rows].T, np.float16)
            im["wvt"] = np.ascontiguousarray(Wv[rows].T, np.float16)
            im["bvr"] = np.broadcast_to(
                bv[rows].astype(np.float16), (P, DC)
            ).copy()
        in_maps.append(im)
    return in_maps


def _assemble(results):
    out = np.empty((B, S, D), np.float32)
    for c in range(8):
        b, g = divmod(c, 2)
        o = results[c]["out"]  # [8, 65, S] f32: rows 0..63 ctxT, row 64 denom
        ctx = o[:, :64, :] / o[:, 64:65, :]
        out[b, :, g * DC : (g + 1) * DC] = ctx.transpose(2, 0, 1).reshape(S, DC)
    return out


def _run(in_maps, trace=False):
    from concourse.bass_utils import run_bass_kernel_spmd

    nc = _get_program()
    return run_bass_kernel_spmd(nc, in_maps, core_ids=list(range(8)), trace=trace)


def kernel(hidden_states, attention_mask, Wq, bq, Wk, bk, Wv, bv):
    in_maps = prepare_in_maps(
        hidden_states, attention_mask, Wq, bq, Wk, bk, Wv, bv
    )
    res = _run(in_maps, trace=False)
    return _assemble(res.results)

